# revision 1
# baseline (speedup 1.0000x reference)
"""GCN encoder kernel for 8 Trainium2 NeuronCores.

Math: out = A_hat @ (x @ (W_gc @ W_fc)) + (b_gc @ W_fc + b_fc), where
A_hat = D^-1/2 (A + I) D^-1/2 (degree over destinations incl self-loops).

Distribution (1D graph parallel, dst-partitioned):
  - nodes sharded 8 ways (12544-padded shards); each core computes its shard
    of g = x @ W2 (W2 = W_gc @ W_fc, host-precomputed on weights only),
  - AllGather of g so every core holds the full [100352, 64] table,
  - each core processes the edges whose destination lands in its shard:
    SWDGE dma_gather of g[src] rows (int16 indices -> 4 source groups of
    25088 rows), per-128-slot-chunk norm-scaled one-hot built on DVE
    (tensor_scalar is_equal*mult), segmented sum via PE matmuls
    accumulating in PSUM (one bank holds 8 destination-tile accumulators),
  - flush adds the bias and DMAs to a [128, 98, 64] partition-major output
    that the host transposes back to [100000, 64].
"""
import numpy as np
import ml_dtypes
from contextlib import ExitStack

N_NODES = 100000
IN_FEAT = 256
OUT_FEAT = 64
NCORES = 8
SHARD = N_NODES // NCORES          # 12500
NTILES = 98                        # ceil(12500/128)
PADSHARD = NTILES * 128            # 12544
GTAB_ROWS = NCORES * PADSHARD      # 100352
NGROUPS = 4
GROUP_ROWS = GTAB_ROWS // NGROUPS  # 25088 (< 32768, int16-addressable)
SUPER_SIZES = [4] * 24 + [2]       # dst tiles per super (sum 98); 4 PSUM banks live
DST_SENTINEL = 255.0               # matches no iota column -> zero one-hot row
DEBUG_STAGE = 3                    # 1: phase1+AG, 2: +phase2 w/o gather, 3: full
MAX_GATHER_IDXS = 1 << 30          # split dma_gather calls to this many indices


def _preprocess(x, edge_index, W_gc, b_gc, W_fc, b_fc):
    """Host-side index/weight preprocessing. Returns per-core input arrays and
    the (input-derived, core-common) chunk schedule."""
    x = np.asarray(x, np.float32)
    W2 = (np.asarray(W_gc, np.float64) @ np.asarray(W_fc, np.float64)).astype(np.float32)
    b_out = (np.asarray(b_gc, np.float64) @ np.asarray(W_fc, np.float64)
             + np.asarray(b_fc, np.float64)).astype(np.float32)

    src = np.asarray(edge_index[0], np.int64)
    dst = np.asarray(edge_index[1], np.int64)
    deg = np.bincount(dst, minlength=N_NODES).astype(np.float64) + 1.0
    dinv = (1.0 / np.sqrt(deg)).astype(np.float32)
    loop = np.arange(N_NODES, dtype=np.int64)
    src = np.concatenate([src, loop])
    dst = np.concatenate([dst, loop])
    norm = dinv[src] * dinv[dst]

    gsrc = (src // SHARD) * PADSHARD + (src % SHARD)       # padded global row
    grp = gsrc // GROUP_ROWS
    lidx = (gsrc % GROUP_ROWS).astype(np.int16)
    core = dst // SHARD
    dloc = dst % SHARD
    tile = dloc // 128
    dst_local = (dloc % 128).astype(np.float32)

    # bucket edges by (core, tile, group)
    key = ((core * NTILES + tile) * NGROUPS + grp).astype(np.int64)
    order = np.argsort(key, kind="stable")
    key_s = key[order]
    lidx_s = lidx[order]
    dstl_s = dst_local[order]
    norm_s = norm[order]
    counts = np.bincount(key_s, minlength=NCORES * NTILES * NGROUPS)
    counts = counts.reshape(NCORES, NTILES, NGROUPS)
    starts = np.zeros_like(counts)
    flat = counts.reshape(NCORES, -1)
    starts.reshape(NCORES, -1)[:, 1:] = np.cumsum(flat, axis=1)[:, :-1]
    starts += np.concatenate([[0], np.cumsum(flat.sum(axis=1))[:-1]]).reshape(-1, 1, 1)

    # core-common schedule: chunks per (tile, group) = max over cores
    chunks_tg = -(-counts.max(axis=0) // 128)              # [NTILES, NGROUPS]

    # slot layout: for super, for group, for tile-in-super
    supers = []
    t0 = 0
    for ssz in SUPER_SIZES:
        supers.append(list(range(t0, t0 + ssz)))
        t0 += ssz

    slot_off = np.zeros((NTILES, NGROUPS), np.int64)       # slot start per (t,g)
    windows = []                                           # per (super, group): (slot0, nchunks)
    pos = 0
    for tiles in supers:
        for g in range(NGROUPS):
            w0 = pos
            for t in tiles:
                slot_off[t, g] = pos
                pos += int(chunks_tg[t, g]) * 128
            windows.append((w0, (pos - w0) // 128))
    S_total = pos
    C_total = S_total // 128

    idx_all = np.zeros((NCORES, S_total), np.int16)
    dst_all = np.full((NCORES, S_total), DST_SENTINEL, np.float32)
    nrm_all = np.zeros((NCORES, S_total), np.float32)
    for c in range(NCORES):
        for t in range(NTILES):
            for g in range(NGROUPS):
                n = int(counts[c, t, g])
                if n == 0:
                    continue
                s0 = int(starts[c, t, g])
                o = int(slot_off[t, g])
                idx_all[c, o:o + n] = lidx_s[s0:s0 + n]
                dst_all[c, o:o + n] = dstl_s[s0:s0 + n]
                nrm_all[c, o:o + n] = norm_s[s0:s0 + n]

    # device layouts
    idx_dev = np.ascontiguousarray(
        np.tile(idx_all.reshape(NCORES, -1, 16).transpose(0, 2, 1), (1, 8, 1)))
    dst_dev = np.ascontiguousarray(
        dst_all.reshape(NCORES, C_total, 128).transpose(0, 2, 1)).astype(ml_dtypes.bfloat16)
    nrm_dev = np.ascontiguousarray(nrm_all.reshape(NCORES, C_total, 128).transpose(0, 2, 1))

    x_pad = np.zeros((NCORES, PADSHARD, IN_FEAT), np.float32)
    x_pad[:, :SHARD] = x.reshape(NCORES, SHARD, IN_FEAT)

    iota_np = np.tile(np.arange(128, dtype=np.float32)[None, :], (128, 1)).astype(ml_dtypes.bfloat16)
    ident_np = np.eye(128, dtype=np.float32)
    bias8_np = np.tile(b_out[None, :], (128, 8)).astype(np.float32)

    # per-tile first/last chunk flags: (group, local chunk j) of first & last chunk
    first_last = []
    for t in range(NTILES):
        gs = [g for g in range(NGROUPS) if chunks_tg[t, g] > 0]
        first_last.append((gs[0], gs[-1]))

    meta = dict(chunks_tg=chunks_tg, supers=supers, windows=windows,
                slot_off=slot_off, S_total=S_total, C_total=C_total,
                first_last=first_last)
    per_core = dict(x=x_pad, idx=idx_dev, dstv=dst_dev, normv=nrm_dev)
    consts = dict(W2=W2, iota=iota_np, ident=ident_np, bias8=bias8_np)
    return meta, per_core, consts


def _build(meta):
    import concourse.bass as bass
    import concourse.tile as tile
    from concourse import bacc, mybir

    chunks_tg = meta["chunks_tg"]
    supers = meta["supers"]
    windows = meta["windows"]
    slot_off = meta["slot_off"]
    S_total = meta["S_total"]
    C_total = meta["C_total"]
    first_last = meta["first_last"]

    nc = bacc.Bacc("TRN2", target_bir_lowering=False, debug=False,
                   num_devices=NCORES, num_swdge_queues=4)
    f32, bf16, i16 = mybir.dt.float32, mybir.dt.bfloat16, mybir.dt.int16

    x_ap = nc.dram_tensor("x_in", [PADSHARD, IN_FEAT], f32, kind="ExternalInput").ap()
    idx_ap = nc.dram_tensor("idx_in", [128, S_total // 16], i16, kind="ExternalInput").ap()
    dst_ap = nc.dram_tensor("dst_in", [128, C_total], bf16, kind="ExternalInput").ap()
    nrm_ap = nc.dram_tensor("nrm_in", [128, C_total], f32, kind="ExternalInput").ap()
    W2_ap = nc.dram_tensor("w2_in", [IN_FEAT, OUT_FEAT], f32, kind="ExternalInput").ap()
    iota_ap = nc.dram_tensor("iota_in", [128, 128], bf16, kind="ExternalInput").ap()
    ident_ap = nc.dram_tensor("ident_in", [128, 128], f32, kind="ExternalInput").ap()
    bias8_ap = nc.dram_tensor("bias8_in", [128, 512], f32, kind="ExternalInput").ap()
    out_ap = nc.dram_tensor("y_out", [128, NTILES, OUT_FEAT], f32, kind="ExternalOutput").ap()

    with tile.TileContext(nc) as tc, ExitStack() as ctx:
        dram = ctx.enter_context(tc.tile_pool(name="dram", bufs=1, space="DRAM"))
        g_c = dram.tile([PADSHARD, OUT_FEAT], f32)
        g_full = dram.tile([GTAB_ROWS, OUT_FEAT], f32)

        cpool = ctx.enter_context(tc.tile_pool(name="consts", bufs=1))
        iota_t = cpool.tile([128, 128], bf16)
        nc.sync.dma_start(iota_t[:], iota_ap[:])
        bias8_t = cpool.tile([128, 512], f32)
        nc.sync.dma_start(bias8_t[:], bias8_ap[:])

        # ---- phase 1: g_c = x_c @ W2 ----
        with tc.tile_pool(name="ph1", bufs=3) as ph1, \
             tc.tile_pool(name="ph1c", bufs=1) as ph1c, \
             tc.tile_pool(name="ph1ps", bufs=2, space="PSUM") as ph1ps, \
             tc.tile_pool(name="gps", bufs=2, space="PSUM") as gps:
            ident_t = ph1c.tile([128, 128], f32)
            nc.sync.dma_start(ident_t[:], ident_ap[:])
            w2_t = ph1c.tile([128, 2, OUT_FEAT], f32)
            nc.sync.dma_start(w2_t[:], W2_ap.rearrange("(k p) f -> p k f", p=128))
            for nt in range(NTILES):
                xt = ph1.tile([128, IN_FEAT], f32, tag="xt")
                nc.sync.dma_start(xt[:], x_ap[nt * 128:(nt + 1) * 128, :])
                gp = gps.tile([128, OUT_FEAT], f32, tag="gp")
                for k in range(2):
                    tp = ph1ps.tile([128, 128], f32, tag=f"tp{k}")
                    nc.tensor.transpose(tp[:], xt[:, k * 128:(k + 1) * 128], ident_t[:])
                    xT = ph1.tile([128, 128], f32, tag=f"xT{k}")
                    nc.scalar.copy(xT[:], tp[:])
                    nc.tensor.matmul(gp[:], xT[:], w2_t[:, k, :],
                                     start=(k == 0), stop=(k == 1))
                gs = ph1.tile([128, OUT_FEAT], f32, tag="gs")
                nc.vector.tensor_copy(gs[:], gp[:])
                nc.sync.dma_start(g_c[nt * 128:(nt + 1) * 128, :], gs[:])

        # ---- allgather g ----
        nc.gpsimd.collective_compute(
            "AllGather", mybir.AluOpType.bypass,
            ins=[g_c.opt()], outs=[g_full.opt()],
            replica_groups=[list(range(NCORES))],
        )

        # ---- phase 2: gather + segmented-sum matmuls ----
        p2 = ctx.enter_context(tc.tile_pool(name="p2", bufs=2))
        psum2 = ctx.enter_context(tc.tile_pool(name="ps2", bufs=2, space="PSUM"))
        outp = ctx.enter_context(tc.tile_pool(name="outp", bufs=2))

        win_i = 0
        for si, tiles in enumerate(supers if DEBUG_STAGE >= 2 else []):
            # load + gather the 4 group windows of this super
            gbuf = {}   # g -> (msg16 tile, dst tile, nrm tile, w0)
            for g in range(NGROUPS):
                w0, wch = windows[win_i]
                win_i += 1
                if wch == 0:
                    continue
                nsl = wch * 128
                idx_t = p2.tile([128, nsl // 16], i16, tag="idx", bufs=4)
                nc.sync.dma_start(idx_t[:], idx_ap[:, w0 // 16:(w0 + nsl) // 16])
                dst_t = p2.tile([128, wch], bf16, tag=f"dst{g}", bufs=3)
                nc.sync.dma_start(dst_t[:], dst_ap[:, w0 // 128:(w0 + nsl) // 128])
                nrm_t = p2.tile([128, wch], f32, tag=f"nrm{g}", bufs=3)
                nc.sync.dma_start(nrm_t[:], nrm_ap[:, w0 // 128:(w0 + nsl) // 128])

                msg32 = p2.tile([128, wch, OUT_FEAT], f32, tag="msg32", bufs=4)
                if DEBUG_STAGE >= 3:
                    nc.gpsimd.dma_gather(
                        msg32[:], g_full[g * GROUP_ROWS:(g + 1) * GROUP_ROWS, :],
                        idx_t[:], wch * 128, wch * 128,
                        OUT_FEAT, single_packet=False, queue_num=(si * 4 + g) % 4,
                    )
                else:
                    nc.vector.memset(msg32[:], 0.0)
                # fused norm-scale + fp32->bf16 cast (replaces ACT copy)
                msg16 = p2.tile([128, wch, OUT_FEAT], bf16, tag=f"msg16{g}", bufs=3)
                nc.vector.tensor_tensor(
                    out=msg16[:], in0=msg32[:],
                    in1=nrm_t[:].unsqueeze(2).broadcast_to([128, wch, OUT_FEAT]),
                    op=mybir.AluOpType.mult)
                # batched one-hot for the whole window (0/1 bf16)
                oh_win = p2.tile([128, wch, 128], bf16, tag=f"oh{g}", bufs=3)
                nc.vector.tensor_tensor(
                    out=oh_win[:],
                    in0=iota_t[:].unsqueeze(1).broadcast_to([128, wch, 128]),
                    in1=dst_t[:].unsqueeze(2).broadcast_to([128, wch, 128]),
                    op=mybir.AluOpType.is_equal)
                gbuf[g] = (msg16, oh_win, w0)

            stg = outp.tile([128, len(tiles), OUT_FEAT], f32, tag="stg")
            # tile-major accumulation: one PSUM bank per in-flight tile
            for ti, t in enumerate(tiles):
                acc = psum2.tile([128, OUT_FEAT], f32, tag=f"acc{ti % 4}",
                                 name=f"acc_{si}_{ti}")
                gfirst, glast = first_last[t]
                for g in range(NGROUPS):
                    ntch = int(chunks_tg[t, g])
                    if ntch == 0:
                        continue
                    msg16, oh_win, w0 = gbuf[g]
                    c0 = (slot_off[t, g] - w0) // 128
                    for j in range(ntch):
                        cj = c0 + j
                        nc.tensor.matmul(
                            acc[:], oh_win[:, cj, :], msg16[:, cj, :],
                            start=(g == gfirst and j == 0),
                            stop=(g == glast and j == ntch - 1),
                        )
                nc.vector.tensor_tensor(
                    out=stg[:, ti, :], in0=acc[:], in1=bias8_t[:, :OUT_FEAT],
                    op=mybir.AluOpType.add,
                )
            nc.sync.dma_start(out_ap[:, tiles[0]:tiles[0] + len(tiles), :], stg[:])

    nc.compile()
    return nc


_CACHED = {}


def kernel(x, edge_index, W_gc, b_gc, W_fc, b_fc):
    from concourse import bass_utils

    meta, per_core, consts = _preprocess(x, edge_index, W_gc, b_gc, W_fc, b_fc)
    cache_key = (meta["S_total"], meta["C_total"],
                 tuple(map(tuple, meta["chunks_tg"])))
    if cache_key in _CACHED:
        nc = _CACHED[cache_key]
    else:
        nc = _build(meta)
        _CACHED.clear()
        _CACHED[cache_key] = nc

    in_maps = []
    for c in range(NCORES):
        in_maps.append({
            "x_in": per_core["x"][c],
            "idx_in": per_core["idx"][c],
            "dst_in": per_core["dstv"][c],
            "nrm_in": per_core["normv"][c],
            "w2_in": consts["W2"],
            "iota_in": consts["iota"],
            "ident_in": consts["ident"],
            "bias8_in": consts["bias8"],
        })
    res = bass_utils.run_bass_kernel_spmd(nc, in_maps, core_ids=list(range(NCORES)))
    out = np.empty((N_NODES, OUT_FEAT), np.float32)
    for c in range(NCORES):
        oc = res.results[c]["y_out"]                      # [128, 98, 64]
        out[c * SHARD:(c + 1) * SHARD] = (
            oc.transpose(1, 0, 2).reshape(PADSHARD, OUT_FEAT)[:SHARD])
    return out



# revision 5
# speedup vs baseline: 1.2880x; 1.2880x over previous
"""GCN encoder kernel for 8 Trainium2 NeuronCores.

Math: out = A_hat @ (x @ (W_gc @ W_fc)) + (b_gc @ W_fc + b_fc), where
A_hat = D^-1/2 (A + I) D^-1/2 (degree over destinations incl self-loops).

Key factorization: norm(e) = dinv[src]*dinv[dst], so
  out[d] = dinv[d] * (sum_{e: dst=d} g'[src(e)]) + b_out,  g' = (dinv*x) @ W2.
The per-edge norm multiply disappears: the src factor is folded into x on the
host, the dst factor is applied once per output tile, and the bias is
injected into the PSUM accumulation as an identity matmul of b_out/dinv[d].

Distribution (1D graph parallel, dst-partitioned):
  - nodes sharded 8 ways; each core computes its shard of g' = x' @ W2 in 4
    row-chunks (x shipped pre-transposed and dinv-prescaled, so phase 1 is
    pure matmul with no PE transpose),
  - 4 chunked AllGathers (one per gather group, Shared outputs) overlap with
    phase-1 compute and phase-2 gathers of earlier groups,
  - each core processes the edges whose destination lands in its shard:
    SWDGE dma_gather of g'[src] rows (int16 indices -> 4 source groups),
    0/1 one-hot built on DVE (is_equal vs iota), msg cast f32->bf16 on the
    Scalar engine, segmented sum via PE matmuls accumulating in PSUM
    (bias'_t injected via identity matmul), flush = ACT copy scaled by
    dinv[dst] straight to the staging tile.
"""
import numpy as np
import ml_dtypes
from contextlib import ExitStack

N_NODES = 100000
IN_FEAT = 256
OUT_FEAT = 64
NCORES = 8
SHARD = N_NODES // NCORES          # 12500
NTILES = 98                        # ceil(12500/128)
PADSHARD = NTILES * 128            # 12544
CH_TILES = [25, 25, 24, 24]        # phase-1 chunks (tiles), one per group
CH_ROWS = [t * 128 for t in CH_TILES]            # [3200,3200,3072,3072]
CH_START = [0, 3200, 6400, 9472]
GRP_ROWS = [r * NCORES for r in CH_ROWS]         # [25600,25600,24576,24576]
GRP_BASE = [0, 25600, 51200, 75776]
NGROUPS = 4
SUPER_SIZES = [4] * 24 + [2]       # dst tiles per super (sum 98)
DST_SENTINEL = 255.0               # matches no iota column -> zero one-hot row


def _preprocess(x, edge_index, W_gc, b_gc, W_fc, b_fc):
    """Host-side index/weight preprocessing. Returns per-core input arrays and
    the (input-derived, core-common) chunk schedule."""
    x = np.asarray(x, np.float32)
    W2 = (np.asarray(W_gc, np.float64) @ np.asarray(W_fc, np.float64)).astype(np.float32)
    b_out = (np.asarray(b_gc, np.float64) @ np.asarray(W_fc, np.float64)
             + np.asarray(b_fc, np.float64))          # [64] f64

    src = np.asarray(edge_index[0], np.int64)
    dst = np.asarray(edge_index[1], np.int64)
    deg = np.bincount(dst, minlength=N_NODES).astype(np.float64) + 1.0
    dinv = 1.0 / np.sqrt(deg)                         # f64 [N]
    loop = np.arange(N_NODES, dtype=np.int64)
    src = np.concatenate([src, loop])
    dst = np.concatenate([dst, loop])

    # x' = dinv[v] * x[v], shipped transposed per core: [256, PADSHARD]
    xs = (x * dinv[:, None].astype(np.float32))
    xT = np.zeros((NCORES, IN_FEAT, PADSHARD), np.float32)
    for c in range(NCORES):
        xT[c, :, :SHARD] = xs[c * SHARD:(c + 1) * SHARD].T

    # source table index: group = phase-1 chunk, row = c*CH_ROWS[k] + local
    sc = src // SHARD
    sr = src % SHARD
    grp = np.searchsorted(np.array(CH_START[1:] + [PADSHARD]), sr, side='right')
    lidx = (sc * np.array(CH_ROWS)[grp] + (sr - np.array(CH_START)[grp])).astype(np.int16)

    core = dst // SHARD
    dloc = dst % SHARD
    tile = dloc // 128
    dst_local = (dloc % 128).astype(np.float32)

    # bucket edges by (core, tile, group)
    key = ((core * NTILES + tile) * NGROUPS + grp).astype(np.int64)
    order = np.argsort(key, kind="stable")
    key_s = key[order]
    lidx_s = lidx[order]
    dstl_s = dst_local[order]
    counts = np.bincount(key_s, minlength=NCORES * NTILES * NGROUPS)
    counts = counts.reshape(NCORES, NTILES, NGROUPS)
    starts = np.zeros_like(counts)
    flat = counts.reshape(NCORES, -1)
    starts.reshape(NCORES, -1)[:, 1:] = np.cumsum(flat, axis=1)[:, :-1]
    starts += np.concatenate([[0], np.cumsum(flat.sum(axis=1))[:-1]]).reshape(-1, 1, 1)

    # core-common schedule: chunks per (tile, group) = max over cores
    chunks_tg = -(-counts.max(axis=0) // 128)              # [NTILES, NGROUPS]

    supers = []
    t0 = 0
    for ssz in SUPER_SIZES:
        supers.append(list(range(t0, t0 + ssz)))
        t0 += ssz

    slot_off = np.zeros((NTILES, NGROUPS), np.int64)       # slot start per (t,g)
    windows = []                                           # per (super, group): (slot0, nchunks)
    pos = 0
    for tiles in supers:
        for g in range(NGROUPS):
            w0 = pos
            for t in tiles:
                slot_off[t, g] = pos
                pos += int(chunks_tg[t, g]) * 128
            windows.append((w0, (pos - w0) // 128))
    S_total = pos
    C_total = S_total // 128

    idx_all = np.zeros((NCORES, S_total), np.int16)
    dst_all = np.full((NCORES, S_total), DST_SENTINEL, np.float32)
    for c in range(NCORES):
        for t in range(NTILES):
            for g in range(NGROUPS):
                n = int(counts[c, t, g])
                if n == 0:
                    continue
                s0 = int(starts[c, t, g])
                o = int(slot_off[t, g])
                idx_all[c, o:o + n] = lidx_s[s0:s0 + n]
                dst_all[c, o:o + n] = dstl_s[s0:s0 + n]
    # device layouts
    idx_dev = np.ascontiguousarray(
        np.tile(idx_all.reshape(NCORES, -1, 16).transpose(0, 2, 1), (1, 8, 1)))
    dst_dev = np.ascontiguousarray(
        dst_all.reshape(NCORES, C_total, 128).transpose(0, 2, 1)).astype(ml_dtypes.bfloat16)

    # dst-side scale and pre-divided bias, per tile
    dinv_pad = np.zeros((NCORES, PADSHARD), np.float64)
    for c in range(NCORES):
        dinv_pad[c, :SHARD] = dinv[c * SHARD:(c + 1) * SHARD]
    dinv_dev = np.ascontiguousarray(
        dinv_pad.reshape(NCORES, NTILES, 128).transpose(0, 2, 1)).astype(np.float32)
    with np.errstate(divide='ignore', invalid='ignore'):
        biasp = b_out[None, None, :] / np.where(dinv_pad > 0, dinv_pad, np.inf)[:, :, None]
    biasp_dev = np.ascontiguousarray(
        biasp.reshape(NCORES, NTILES, 128, OUT_FEAT).transpose(0, 2, 1, 3)
    ).astype(ml_dtypes.bfloat16)                           # [NC,128,NTILES,64]

    iota_np = np.tile(np.arange(128, dtype=np.float32)[None, :], (128, 1)).astype(ml_dtypes.bfloat16)
    ident_np = np.eye(128, dtype=np.float32).astype(ml_dtypes.bfloat16)

    # per-tile first/last chunk flags
    first_last = []
    for t in range(NTILES):
        gs = [g for g in range(NGROUPS) if chunks_tg[t, g] > 0]
        first_last.append((gs[0], gs[-1]) if gs else (None, None))

    meta = dict(chunks_tg=chunks_tg, supers=supers, windows=windows,
                slot_off=slot_off, S_total=S_total, C_total=C_total,
                first_last=first_last)
    per_core = dict(xT=xT, idx=idx_dev, dstv=dst_dev, dinv=dinv_dev, biasp=biasp_dev)
    consts = dict(W2=W2, iota=iota_np, ident=ident_np)
    return meta, per_core, consts


def _build(meta):
    import concourse.bass as bass
    import concourse.tile as tile
    from concourse import bacc, mybir

    chunks_tg = meta["chunks_tg"]
    supers = meta["supers"]
    windows = meta["windows"]
    slot_off = meta["slot_off"]
    S_total = meta["S_total"]
    C_total = meta["C_total"]
    first_last = meta["first_last"]

    nc = bacc.Bacc("TRN2", target_bir_lowering=False, debug=False,
                   num_devices=NCORES, num_swdge_queues=4)
    f32, bf16, i16 = mybir.dt.float32, mybir.dt.bfloat16, mybir.dt.int16

    xT_ap = nc.dram_tensor("xt_in", [IN_FEAT, PADSHARD], f32, kind="ExternalInput").ap()
    idx_ap = nc.dram_tensor("idx_in", [128, S_total // 16], i16, kind="ExternalInput").ap()
    dst_ap = nc.dram_tensor("dst_in", [128, C_total], bf16, kind="ExternalInput").ap()
    dinv_ap = nc.dram_tensor("dinv_in", [128, NTILES], f32, kind="ExternalInput").ap()
    biasp_ap = nc.dram_tensor("biasp_in", [128, NTILES, OUT_FEAT], bf16, kind="ExternalInput").ap()
    W2_ap = nc.dram_tensor("w2_in", [IN_FEAT, OUT_FEAT], f32, kind="ExternalInput").ap()
    iota_ap = nc.dram_tensor("iota_in", [128, 128], bf16, kind="ExternalInput").ap()
    ident_ap = nc.dram_tensor("ident_in", [128, 128], bf16, kind="ExternalInput").ap()
    out_ap = nc.dram_tensor("y_out", [128, NTILES, OUT_FEAT], f32, kind="ExternalOutput").ap()

    with tile.TileContext(nc) as tc, ExitStack() as ctx:
        dram = ctx.enter_context(tc.tile_pool(name="dram", bufs=1, space="DRAM"))
        g_c = [dram.tile([CH_ROWS[k], OUT_FEAT], f32, name=f"g_c{k}")
               for k in range(4)]
        g_full = [dram.tile([GRP_ROWS[k], OUT_FEAT], f32, name=f"g_full{k}",
                            addr_space="Shared")
                  for k in range(4)]

        cpool = ctx.enter_context(tc.tile_pool(name="consts", bufs=1))
        iota_t = cpool.tile([128, 128], bf16)
        nc.sync.dma_start(iota_t[:], iota_ap[:])
        ident_t = cpool.tile([128, 128], bf16)
        nc.sync.dma_start(ident_t[:], ident_ap[:])
        dinv_t = cpool.tile([128, NTILES], f32)
        nc.sync.dma_start(dinv_t[:], dinv_ap[:])
        biasp_t = cpool.tile([128, NTILES, OUT_FEAT], bf16)
        nc.sync.dma_start(biasp_t[:], biasp_ap[:])
        w2_t = cpool.tile([128, 2, OUT_FEAT], f32)
        nc.sync.dma_start(w2_t[:], W2_ap.rearrange("(k p) f -> p k f", p=128))

        # ---- phase 1: g'_c = x'_c @ W2, 4 chunks, chunked AllGather ----
        with tc.tile_pool(name="ph1", bufs=2) as ph1, \
             tc.tile_pool(name="ph1s", bufs=4) as ph1s, \
             tc.tile_pool(name="gps", bufs=2, space="PSUM") as gps:
            for k in range(4):
                c0 = CH_START[k] * 1
                ncols = CH_ROWS[k]
                xt0 = ph1.tile([128, ncols], f32, tag="xt0")
                nc.sync.dma_start(xt0[:], xT_ap[0:128, c0:c0 + ncols])
                xt1 = ph1.tile([128, ncols], f32, tag="xt1")
                nc.sync.dma_start(xt1[:], xT_ap[128:256, c0:c0 + ncols])
                for i in range(CH_TILES[k]):
                    gp = gps.tile([128, OUT_FEAT], f32, tag="gp")
                    nc.tensor.matmul(gp[:], xt0[:, i * 128:(i + 1) * 128],
                                     w2_t[:, 0, :], start=True, stop=False)
                    nc.tensor.matmul(gp[:], xt1[:, i * 128:(i + 1) * 128],
                                     w2_t[:, 1, :], start=False, stop=True)
                    gs = ph1s.tile([128, OUT_FEAT], f32, tag="gs")
                    nc.scalar.copy(gs[:], gp[:])
                    nc.sync.dma_start(g_c[k][i * 128:(i + 1) * 128, :], gs[:])
                nc.gpsimd.collective_compute(
                    "AllGather", mybir.AluOpType.bypass,
                    ins=[g_c[k].opt()], outs=[g_full[k].opt()],
                    replica_groups=[list(range(NCORES))],
                )

        # ---- phase 2: gather + one-hot + segmented-sum matmuls ----
        p2 = ctx.enter_context(tc.tile_pool(name="p2", bufs=2))
        psum2 = ctx.enter_context(tc.tile_pool(name="ps2", bufs=2, space="PSUM"))
        outp = ctx.enter_context(tc.tile_pool(name="outp", bufs=3))

        win_i = 0
        for si, tiles in enumerate(supers):
            gbuf = {}   # g -> (msg16, oh_win, w0)
            for g in range(NGROUPS):
                w0, wch = windows[win_i]
                win_i += 1
                if wch == 0:
                    continue
                nsl = wch * 128
                idx_t = p2.tile([128, nsl // 16], i16, tag="idx", bufs=4)
                nc.sync.dma_start(idx_t[:], idx_ap[:, w0 // 16:(w0 + nsl) // 16])
                dst_t = p2.tile([128, wch], bf16, tag="dst", bufs=4)
                nc.sync.dma_start(dst_t[:], dst_ap[:, w0 // 128:(w0 + nsl) // 128])

                msg32 = p2.tile([128, wch, OUT_FEAT], f32, tag="msg32", bufs=3)
                nc.gpsimd.dma_gather(
                    msg32[:], g_full[g][:, :],
                    idx_t[:], nsl, nsl,
                    OUT_FEAT, single_packet=False, queue_num=(si * 4 + g) % 4,
                )
                # f32 -> bf16 cast on the Scalar engine (frees DVE)
                msg16 = p2.tile([128, wch, OUT_FEAT], bf16, tag="msg16", bufs=8)
                nc.scalar.copy(msg16[:], msg32[:])
                # batched 0/1 one-hot for the whole window
                oh_win = p2.tile([128, wch, 128], bf16, tag="oh", bufs=8)
                nc.vector.tensor_tensor(
                    out=oh_win[:],
                    in0=iota_t[:].unsqueeze(1).broadcast_to([128, wch, 128]),
                    in1=dst_t[:].unsqueeze(2).broadcast_to([128, wch, 128]),
                    op=mybir.AluOpType.is_equal)
                gbuf[g] = (msg16, oh_win, w0)

            stg = outp.tile([128, len(tiles), OUT_FEAT], f32, tag="stg")
            for ti, t in enumerate(tiles):
                acc = psum2.tile([128, OUT_FEAT], f32, tag=f"acc{ti % 4}",
                                 name=f"acc_{si}_{ti}")
                # bias'(t) = b_out / dinv[dst], injected via identity matmul
                nc.tensor.matmul(acc[:], ident_t[:], biasp_t[:, t, :],
                                 start=True, stop=False)
                gfirst, glast = first_last[t]
                for g in range(NGROUPS):
                    ntch = int(chunks_tg[t, g])
                    if ntch == 0:
                        continue
                    msg16, oh_win, w0 = gbuf[g]
                    c0 = (slot_off[t, g] - w0) // 128
                    for j in range(ntch):
                        cj = c0 + j
                        nc.tensor.matmul(
                            acc[:], oh_win[:, cj, :], msg16[:, cj, :],
                            start=False,
                            stop=(g == glast and j == ntch - 1),
                        )
                # flush: out = dinv[dst] * acc   (bias already inside acc)
                nc.scalar.activation(
                    stg[:, ti, :], acc[:],
                    mybir.ActivationFunctionType.Copy,
                    scale=dinv_t[:, t:t + 1])
            nc.sync.dma_start(out_ap[:, tiles[0]:tiles[0] + len(tiles), :], stg[:])

    nc.compile()
    return nc


_CACHED = {}


def make_in_maps(per_core, consts):
    in_maps = []
    for c in range(NCORES):
        in_maps.append({
            "xt_in": per_core["xT"][c],
            "idx_in": per_core["idx"][c],
            "dst_in": per_core["dstv"][c],
            "dinv_in": per_core["dinv"][c],
            "biasp_in": per_core["biasp"][c],
            "w2_in": consts["W2"],
            "iota_in": consts["iota"],
            "ident_in": consts["ident"],
        })
    return in_maps


def kernel(x, edge_index, W_gc, b_gc, W_fc, b_fc):
    from concourse import bass_utils

    meta, per_core, consts = _preprocess(x, edge_index, W_gc, b_gc, W_fc, b_fc)
    cache_key = (meta["S_total"], meta["C_total"],
                 tuple(map(tuple, meta["chunks_tg"])))
    if cache_key in _CACHED:
        nc = _CACHED[cache_key]
    else:
        nc = _build(meta)
        _CACHED.clear()
        _CACHED[cache_key] = nc

    in_maps = make_in_maps(per_core, consts)
    res = bass_utils.run_bass_kernel_spmd(nc, in_maps, core_ids=list(range(NCORES)))
    out = np.empty((N_NODES, OUT_FEAT), np.float32)
    for c in range(NCORES):
        oc = res.results[c]["y_out"]                      # [128, 98, 64]
        out[c * SHARD:(c + 1) * SHARD] = (
            oc.transpose(1, 0, 2).reshape(PADSHARD, OUT_FEAT)[:SHARD])
    return out


# revision 9
# speedup vs baseline: 1.2920x; 1.0031x over previous
"""GCN encoder kernel for 8 Trainium2 NeuronCores.

Math: out = A_hat @ (x @ (W_gc @ W_fc)) + (b_gc @ W_fc + b_fc), where
A_hat = D^-1/2 (A + I) D^-1/2 (degree over destinations incl self-loops).

Key factorization: norm(e) = dinv[src]*dinv[dst], so
  out[d] = dinv[d] * (sum_{e: dst=d} g'[src(e)]) + b_out,  g' = (dinv*x) @ W2.
The per-edge norm multiply disappears: the src factor is folded into x on the
host, the dst factor is applied once per output tile, and the bias is
injected into the PSUM accumulation as an identity matmul of b_out/dinv[d].

Distribution (1D graph parallel, dst-partitioned):
  - nodes sharded 8 ways; each core computes its shard of g' = x' @ W2 in 4
    row-chunks (x shipped pre-transposed and dinv-prescaled, so phase 1 is
    pure matmul with no PE transpose),
  - 4 chunked AllGathers (one per gather group, Shared outputs) overlap with
    phase-1 compute and phase-2 gathers of earlier groups,
  - each core processes the edges whose destination lands in its shard:
    SWDGE dma_gather of g'[src] rows (int16 indices -> 4 source groups),
    0/1 one-hot built on DVE (is_equal vs iota), msg cast f32->bf16 on the
    Scalar engine, segmented sum via PE matmuls accumulating in PSUM
    (bias'_t injected via identity matmul), flush = ACT copy scaled by
    dinv[dst] straight to the staging tile.
"""
import numpy as np
import ml_dtypes
from contextlib import ExitStack

N_NODES = 100000
IN_FEAT = 256
OUT_FEAT = 64
NCORES = 8
SHARD = N_NODES // NCORES          # 12500
NTILES = 98                        # ceil(12500/128)
PADSHARD = NTILES * 128            # 12544
CH_TILES = [25, 25, 24, 24]        # phase-1 chunks (tiles), one per group
CH_ROWS = [t * 128 for t in CH_TILES]            # [3200,3200,3072,3072]
CH_START = [0, 3200, 6400, 9472]
GRP_ROWS = [r * NCORES for r in CH_ROWS]         # [25600,25600,24576,24576]
GRP_BASE = [0, 25600, 51200, 75776]
NGROUPS = 4
SUPER_SIZES = [4] * 24 + [2]       # dst tiles per super (sum 98)
DST_SENTINEL = 255.0               # matches no iota column -> zero one-hot row


def _preprocess(x, edge_index, W_gc, b_gc, W_fc, b_fc):
    """Host-side index/weight preprocessing. Returns per-core input arrays and
    the (input-derived, core-common) chunk schedule."""
    x = np.asarray(x, np.float32)
    W2 = (np.asarray(W_gc, np.float64) @ np.asarray(W_fc, np.float64)).astype(np.float32)
    b_out = (np.asarray(b_gc, np.float64) @ np.asarray(W_fc, np.float64)
             + np.asarray(b_fc, np.float64))          # [64] f64

    src = np.asarray(edge_index[0], np.int64)
    dst = np.asarray(edge_index[1], np.int64)
    deg = np.bincount(dst, minlength=N_NODES).astype(np.float64) + 1.0
    dinv = 1.0 / np.sqrt(deg)                         # f64 [N]
    loop = np.arange(N_NODES, dtype=np.int64)
    src = np.concatenate([src, loop])
    dst = np.concatenate([dst, loop])

    # x' = dinv[v] * x[v], shipped transposed per core: [256, PADSHARD]
    xs = (x * dinv[:, None].astype(np.float32))
    xT = np.zeros((NCORES, IN_FEAT, PADSHARD), np.float32)
    for c in range(NCORES):
        xT[c, :, :SHARD] = xs[c * SHARD:(c + 1) * SHARD].T

    # source table index: group = phase-1 chunk, row = c*CH_ROWS[k] + local
    sc = src // SHARD
    sr = src % SHARD
    grp = np.searchsorted(np.array(CH_START[1:] + [PADSHARD]), sr, side='right')
    lidx = (sc * np.array(CH_ROWS)[grp] + (sr - np.array(CH_START)[grp])).astype(np.int16)

    core = dst // SHARD
    dloc = dst % SHARD
    tile = dloc // 128
    dst_local = (dloc % 128).astype(np.float32)

    # bucket edges by (core, tile, group)
    key = ((core * NTILES + tile) * NGROUPS + grp).astype(np.int64)
    order = np.argsort(key, kind="stable")
    key_s = key[order]
    lidx_s = lidx[order]
    dstl_s = dst_local[order]
    counts = np.bincount(key_s, minlength=NCORES * NTILES * NGROUPS)
    counts = counts.reshape(NCORES, NTILES, NGROUPS)
    starts = np.zeros_like(counts)
    flat = counts.reshape(NCORES, -1)
    starts.reshape(NCORES, -1)[:, 1:] = np.cumsum(flat, axis=1)[:, :-1]
    starts += np.concatenate([[0], np.cumsum(flat.sum(axis=1))[:-1]]).reshape(-1, 1, 1)

    # core-common schedule: chunks per (tile, group) = max over cores
    chunks_tg = -(-counts.max(axis=0) // 128)              # [NTILES, NGROUPS]

    supers = []
    t0 = 0
    for ssz in SUPER_SIZES:
        supers.append(list(range(t0, t0 + ssz)))
        t0 += ssz

    slot_off = np.zeros((NTILES, NGROUPS), np.int64)       # slot start per (t,g)
    windows = []                                           # per (super, group): (slot0, nchunks)
    pos = 0
    for tiles in supers:
        for g in range(NGROUPS):
            w0 = pos
            for t in tiles:
                slot_off[t, g] = pos
                pos += int(chunks_tg[t, g]) * 128
            windows.append((w0, (pos - w0) // 128))
    S_total = pos
    C_total = S_total // 128

    idx_all = np.zeros((NCORES, S_total), np.int16)
    dst_all = np.full((NCORES, S_total), DST_SENTINEL, np.float32)
    for c in range(NCORES):
        for t in range(NTILES):
            for g in range(NGROUPS):
                n = int(counts[c, t, g])
                if n == 0:
                    continue
                s0 = int(starts[c, t, g])
                o = int(slot_off[t, g])
                idx_all[c, o:o + n] = lidx_s[s0:s0 + n]
                dst_all[c, o:o + n] = dstl_s[s0:s0 + n]
    # device layouts
    idx_dev = np.ascontiguousarray(
        np.tile(idx_all.reshape(NCORES, -1, 16).transpose(0, 2, 1), (1, 8, 1)))
    dst_dev = np.ascontiguousarray(
        dst_all.reshape(NCORES, C_total, 128).transpose(0, 2, 1)).astype(ml_dtypes.bfloat16)

    # dst-side scale and pre-divided bias, per tile
    dinv_pad = np.zeros((NCORES, PADSHARD), np.float64)
    for c in range(NCORES):
        dinv_pad[c, :SHARD] = dinv[c * SHARD:(c + 1) * SHARD]
    dinv_dev = np.ascontiguousarray(
        dinv_pad.reshape(NCORES, NTILES, 128).transpose(0, 2, 1)).astype(np.float32)
    with np.errstate(divide='ignore', invalid='ignore'):
        biasp = b_out[None, None, :] / np.where(dinv_pad > 0, dinv_pad, np.inf)[:, :, None]
    biasp_dev = np.ascontiguousarray(
        biasp.reshape(NCORES, NTILES, 128, OUT_FEAT).transpose(0, 2, 1, 3)
    ).astype(ml_dtypes.bfloat16)                           # [NC,128,NTILES,64]

    iota_np = np.tile(np.arange(128, dtype=np.float32)[None, :], (128, 1)).astype(ml_dtypes.bfloat16)
    ident_np = np.eye(128, dtype=np.float32).astype(ml_dtypes.bfloat16)

    # per-tile first/last chunk flags
    first_last = []
    for t in range(NTILES):
        gs = [g for g in range(NGROUPS) if chunks_tg[t, g] > 0]
        first_last.append((gs[0], gs[-1]) if gs else (None, None))

    meta = dict(chunks_tg=chunks_tg, supers=supers, windows=windows,
                slot_off=slot_off, S_total=S_total, C_total=C_total,
                first_last=first_last)
    per_core = dict(xT=xT, idx=idx_dev, dstv=dst_dev, dinv=dinv_dev, biasp=biasp_dev)
    consts = dict(W2=W2, iota=iota_np, ident=ident_np)
    return meta, per_core, consts


def _emit_gather_128b(nc, mybir, out_ap, in_ap, idxs_ap, num_idxs,
                      elem_size, elem_step, queue_num):
    """dma_gather with a 128-byte payload from 256B-strided rows.

    Replica of BassGpSimd.dma_gather's non-transpose path minus the
    python-level `elem_size_bytes % 256 == 0` check: the ISA only requires
    the row *stride* to be a 256B multiple (stride_bytes_256 field); the
    payload per descriptor can be any packet size.
    """
    eng = nc.gpsimd
    stride_bytes = elem_step * mybir.dt.size(in_ap.dtype)
    stride_bytes_256 = stride_bytes // 256
    assert stride_bytes % 256 == 0 and stride_bytes_256 < 256
    _in_ap = eng.lower_ap_dma(in_ap, for_custom_bir_dma=True)
    _idxs_ap = eng.lower_ap(idxs_ap)
    _out_ap = eng.lower_ap(out_ap)
    return eng.add_instruction(
        mybir.InstDMAGatherAnt(
            name=nc.get_next_instruction_name(),
            ins=[*_in_ap, _idxs_ap,
                 eng.lower_val_access(eng.to_reg(num_idxs))],
            outs=[_out_ap],
            transpose=False,
            num_idxs=num_idxs,
            elem_size=elem_size,
            stride_bytes_256=stride_bytes_256,
            gen_mode=0,
            single_packet=False,
            queue_num=queue_num,
            sbuf_tokens_per_rank=0,
            sbuf_free_dim_per_rank=0,
            sbuf_free_dim_pad_per_rank=0,
            sbuf_byte_offset=0,
        )
    )


def _build(meta):
    import concourse.bass as bass
    import concourse.tile as tile
    from concourse import bacc, mybir

    chunks_tg = meta["chunks_tg"]
    supers = meta["supers"]
    windows = meta["windows"]
    slot_off = meta["slot_off"]
    S_total = meta["S_total"]
    C_total = meta["C_total"]
    first_last = meta["first_last"]

    nc = bacc.Bacc("TRN2", target_bir_lowering=False, debug=False,
                   num_devices=NCORES, num_swdge_queues=4)
    f32, bf16, i16 = mybir.dt.float32, mybir.dt.bfloat16, mybir.dt.int16

    xT_ap = nc.dram_tensor("xt_in", [IN_FEAT, PADSHARD], f32, kind="ExternalInput").ap()
    idx_ap = nc.dram_tensor("idx_in", [128, S_total // 16], i16, kind="ExternalInput").ap()
    dst_ap = nc.dram_tensor("dst_in", [128, C_total], bf16, kind="ExternalInput").ap()
    dinv_ap = nc.dram_tensor("dinv_in", [128, NTILES], f32, kind="ExternalInput").ap()
    biasp_ap = nc.dram_tensor("biasp_in", [128, NTILES, OUT_FEAT], bf16, kind="ExternalInput").ap()
    W2_ap = nc.dram_tensor("w2_in", [IN_FEAT, OUT_FEAT], f32, kind="ExternalInput").ap()
    iota_ap = nc.dram_tensor("iota_in", [128, 128], bf16, kind="ExternalInput").ap()
    ident_ap = nc.dram_tensor("ident_in", [128, 128], bf16, kind="ExternalInput").ap()
    out_ap = nc.dram_tensor("y_out", [128, NTILES, OUT_FEAT], f32, kind="ExternalOutput").ap()

    with tile.TileContext(nc) as tc, ExitStack() as ctx:
        # g' tables in bf16, rows padded to 128 elems (256B DMA row stride);
        # upper 64 columns are never read.
        dram = ctx.enter_context(tc.tile_pool(name="dram", bufs=1, space="DRAM"))
        g_c = [dram.tile([CH_ROWS[k], 128], bf16, name=f"g_c{k}")
               for k in range(4)]
        g_full = [dram.tile([GRP_ROWS[k], 128], bf16, name=f"g_full{k}",
                            addr_space="Shared")
                  for k in range(4)]

        cpool = ctx.enter_context(tc.tile_pool(name="consts", bufs=1))
        iota_t = cpool.tile([128, 128], bf16)
        nc.sync.dma_start(iota_t[:], iota_ap[:])
        ident_t = cpool.tile([128, 128], bf16)
        nc.sync.dma_start(ident_t[:], ident_ap[:])
        dinv_t = cpool.tile([128, NTILES], f32)
        nc.sync.dma_start(dinv_t[:], dinv_ap[:])
        biasp_t = cpool.tile([128, NTILES, OUT_FEAT], bf16)
        nc.sync.dma_start(biasp_t[:], biasp_ap[:])
        w2_t = cpool.tile([128, 2, OUT_FEAT], f32)
        nc.sync.dma_start(w2_t[:], W2_ap.rearrange("(k p) f -> p k f", p=128))

        # ---- phase 1: g'_c = x'_c @ W2, 4 chunks, chunked AllGather ----
        with tc.tile_pool(name="ph1", bufs=2) as ph1, \
             tc.tile_pool(name="ph1s", bufs=4) as ph1s, \
             tc.tile_pool(name="gps", bufs=2, space="PSUM") as gps:
            for k in range(4):
                c0 = CH_START[k] * 1
                ncols = CH_ROWS[k]
                xt0 = ph1.tile([128, ncols], f32, tag="xt0")
                nc.sync.dma_start(xt0[:], xT_ap[0:128, c0:c0 + ncols])
                xt1 = ph1.tile([128, ncols], f32, tag="xt1")
                nc.sync.dma_start(xt1[:], xT_ap[128:256, c0:c0 + ncols])
                for i in range(CH_TILES[k]):
                    gp = gps.tile([128, OUT_FEAT], f32, tag="gp")
                    nc.tensor.matmul(gp[:], xt0[:, i * 128:(i + 1) * 128],
                                     w2_t[:, 0, :], start=True, stop=False)
                    nc.tensor.matmul(gp[:], xt1[:, i * 128:(i + 1) * 128],
                                     w2_t[:, 1, :], start=False, stop=True)
                    gs = ph1s.tile([128, 128], bf16, tag="gs")
                    nc.scalar.copy(gs[:, 0:OUT_FEAT], gp[:])
                    nc.vector.memset(gs[:, OUT_FEAT:128], 0.0)
                    nc.sync.dma_start(g_c[k][i * 128:(i + 1) * 128, :], gs[:])
                nc.gpsimd.collective_compute(
                    "AllGather", mybir.AluOpType.bypass,
                    ins=[g_c[k].opt()], outs=[g_full[k].opt()],
                    replica_groups=[list(range(NCORES))],
                )

        # ---- phase 2: gather + one-hot + segmented-sum matmuls ----
        p2 = ctx.enter_context(tc.tile_pool(name="p2", bufs=2))
        psum2 = ctx.enter_context(tc.tile_pool(name="ps2", bufs=2, space="PSUM"))
        outp = ctx.enter_context(tc.tile_pool(name="outp", bufs=3))

        win_i = 0
        for si, tiles in enumerate(supers):
            gbuf = {}   # g -> (msg16, oh_win, w0)
            for g in range(NGROUPS):
                w0, wch = windows[win_i]
                win_i += 1
                if wch == 0:
                    continue
                nsl = wch * 128
                idx_t = p2.tile([128, nsl // 16], i16, tag="idx", bufs=4)
                nc.sync.dma_start(idx_t[:], idx_ap[:, w0 // 16:(w0 + nsl) // 16])
                dst_t = p2.tile([128, wch], bf16, tag="dst", bufs=4)
                nc.sync.dma_start(dst_t[:], dst_ap[:, w0 // 128:(w0 + nsl) // 128])

                msg16 = p2.tile([128, wch, OUT_FEAT], bf16, tag="msg16", bufs=8)
                _emit_gather_128b(
                    nc, mybir, msg16[:], g_full[g][:, :], idx_t[:],
                    nsl, OUT_FEAT, 128, (si * 4 + g) % 4,
                )
                # batched 0/1 one-hot for the whole window
                oh_win = p2.tile([128, wch, 128], bf16, tag="oh", bufs=8)
                nc.vector.tensor_tensor(
                    out=oh_win[:],
                    in0=iota_t[:].unsqueeze(1).broadcast_to([128, wch, 128]),
                    in1=dst_t[:].unsqueeze(2).broadcast_to([128, wch, 128]),
                    op=mybir.AluOpType.is_equal)
                gbuf[g] = (msg16, oh_win, w0)

            stg = outp.tile([128, len(tiles), OUT_FEAT], f32, tag="stg")
            for ti, t in enumerate(tiles):
                acc = psum2.tile([128, OUT_FEAT], f32, tag=f"acc{ti % 4}",
                                 name=f"acc_{si}_{ti}")
                # bias'(t) = b_out / dinv[dst], injected via identity matmul
                nc.tensor.matmul(acc[:], ident_t[:], biasp_t[:, t, :],
                                 start=True, stop=False)
                gfirst, glast = first_last[t]
                for g in range(NGROUPS):
                    ntch = int(chunks_tg[t, g])
                    if ntch == 0:
                        continue
                    msg16, oh_win, w0 = gbuf[g]
                    c0 = (slot_off[t, g] - w0) // 128
                    for j in range(ntch):
                        cj = c0 + j
                        nc.tensor.matmul(
                            acc[:], oh_win[:, cj, :], msg16[:, cj, :],
                            start=False,
                            stop=(g == glast and j == ntch - 1),
                        )
                # flush: out = dinv[dst] * acc   (bias already inside acc)
                nc.scalar.activation(
                    stg[:, ti, :], acc[:],
                    mybir.ActivationFunctionType.Copy,
                    scale=dinv_t[:, t:t + 1])
            nc.sync.dma_start(out_ap[:, tiles[0]:tiles[0] + len(tiles), :], stg[:])

    nc.compile()
    return nc


_CACHED = {}


def make_in_maps(per_core, consts):
    in_maps = []
    for c in range(NCORES):
        in_maps.append({
            "xt_in": per_core["xT"][c],
            "idx_in": per_core["idx"][c],
            "dst_in": per_core["dstv"][c],
            "dinv_in": per_core["dinv"][c],
            "biasp_in": per_core["biasp"][c],
            "w2_in": consts["W2"],
            "iota_in": consts["iota"],
            "ident_in": consts["ident"],
        })
    return in_maps


def kernel(x, edge_index, W_gc, b_gc, W_fc, b_fc):
    from concourse import bass_utils

    meta, per_core, consts = _preprocess(x, edge_index, W_gc, b_gc, W_fc, b_fc)
    cache_key = (meta["S_total"], meta["C_total"],
                 tuple(map(tuple, meta["chunks_tg"])))
    if cache_key in _CACHED:
        nc = _CACHED[cache_key]
    else:
        nc = _build(meta)
        _CACHED.clear()
        _CACHED[cache_key] = nc

    in_maps = make_in_maps(per_core, consts)
    res = bass_utils.run_bass_kernel_spmd(nc, in_maps, core_ids=list(range(NCORES)))
    out = np.empty((N_NODES, OUT_FEAT), np.float32)
    for c in range(NCORES):
        oc = res.results[c]["y_out"]                      # [128, 98, 64]
        out[c * SHARD:(c + 1) * SHARD] = (
            oc.transpose(1, 0, 2).reshape(PADSHARD, OUT_FEAT)[:SHARD])
    return out


# revision 11
# speedup vs baseline: 1.6724x; 1.2944x over previous
"""GCN encoder kernel for 8 Trainium2 NeuronCores.

Math: out = A_hat @ (x @ (W_gc @ W_fc)) + (b_gc @ W_fc + b_fc), where
A_hat = D^-1/2 (A + I) D^-1/2 (degree over destinations incl self-loops).

Key factorization: norm(e) = dinv[src]*dinv[dst], so
  out[d] = dinv[d] * (sum_{e: dst=d} g'[src(e)]) + b_out,  g' = (dinv*x) @ W2.
The per-edge norm multiply disappears: the src factor is folded into x on the
host, the dst factor is applied once per output tile, and the bias is
injected into the PSUM accumulation as an identity matmul of b_out/dinv[d].

Distribution (1D graph parallel, dst-partitioned):
  - nodes sharded 8 ways; each core computes its shard of g' = x' @ W2 in 4
    row-chunks (x shipped pre-transposed and dinv-prescaled, so phase 1 is
    pure matmul with no PE transpose),
  - 4 chunked AllGathers (one per gather group, Shared outputs) overlap with
    phase-1 compute and phase-2 gathers of earlier groups,
  - each core processes the edges whose destination lands in its shard:
    SWDGE dma_gather of g'[src] rows (int16 indices -> 4 source groups),
    0/1 one-hot built on DVE (is_equal vs iota), msg cast f32->bf16 on the
    Scalar engine, segmented sum via PE matmuls accumulating in PSUM
    (bias'_t injected via identity matmul), flush = ACT copy scaled by
    dinv[dst] straight to the staging tile.
"""
import numpy as np
import ml_dtypes
from contextlib import ExitStack

N_NODES = 100000
IN_FEAT = 256
OUT_FEAT = 64
NCORES = 8
SHARD = N_NODES // NCORES          # 12500
NTILES = 98                        # ceil(12500/128)
PADSHARD = NTILES * 128            # 12544
CH_TILES = [25, 25, 24, 24]        # phase-1 chunks (tiles), one per group
CH_ROWS = [t * 128 for t in CH_TILES]            # [3200,3200,3072,3072]
CH_START = [0, 3200, 6400, 9472]
GRP_ROWS = [r * NCORES for r in CH_ROWS]         # [25600,25600,24576,24576]
GRP_BASE = [0, 25600, 51200, 75776]
NGROUPS = 4
SUPER_SIZES = [4] * 24 + [2]       # dst tiles per super (sum 98)
DST_SENTINEL = 255.0               # matches no iota column -> zero one-hot row


def _preprocess(x, edge_index, W_gc, b_gc, W_fc, b_fc):
    """Host-side index/weight preprocessing. Returns per-core input arrays and
    the (input-derived, core-common) chunk schedule."""
    x = np.asarray(x, np.float32)
    W2 = (np.asarray(W_gc, np.float64) @ np.asarray(W_fc, np.float64)).astype(np.float32)
    b_out = (np.asarray(b_gc, np.float64) @ np.asarray(W_fc, np.float64)
             + np.asarray(b_fc, np.float64))          # [64] f64

    src = np.asarray(edge_index[0], np.int64)
    dst = np.asarray(edge_index[1], np.int64)
    deg = np.bincount(dst, minlength=N_NODES).astype(np.float64) + 1.0
    dinv = 1.0 / np.sqrt(deg)                         # f64 [N]
    loop = np.arange(N_NODES, dtype=np.int64)
    src = np.concatenate([src, loop])
    dst = np.concatenate([dst, loop])

    # x' = dinv[v] * x[v], shipped transposed per core: [256, PADSHARD]
    xs = (x * dinv[:, None].astype(np.float32))
    xT = np.zeros((NCORES, IN_FEAT, PADSHARD), np.float32)
    for c in range(NCORES):
        xT[c, :, :SHARD] = xs[c * SHARD:(c + 1) * SHARD].T

    # source table index: group = phase-1 chunk, row = c*CH_ROWS[k] + local
    sc = src // SHARD
    sr = src % SHARD
    grp = np.searchsorted(np.array(CH_START[1:] + [PADSHARD]), sr, side='right')
    lidx = (sc * np.array(CH_ROWS)[grp] + (sr - np.array(CH_START)[grp])).astype(np.int16)

    core = dst // SHARD
    dloc = dst % SHARD
    tile = dloc // 128
    dst_local = (dloc % 128).astype(np.float32)

    # bucket edges by (core, tile, group)
    key = ((core * NTILES + tile) * NGROUPS + grp).astype(np.int64)
    order = np.argsort(key, kind="stable")
    key_s = key[order]
    lidx_s = lidx[order]
    dstl_s = dst_local[order]
    counts = np.bincount(key_s, minlength=NCORES * NTILES * NGROUPS)
    counts = counts.reshape(NCORES, NTILES, NGROUPS)
    starts = np.zeros_like(counts)
    flat = counts.reshape(NCORES, -1)
    starts.reshape(NCORES, -1)[:, 1:] = np.cumsum(flat, axis=1)[:, :-1]
    starts += np.concatenate([[0], np.cumsum(flat.sum(axis=1))[:-1]]).reshape(-1, 1, 1)

    # core-common schedule: chunks per (tile, group) = max over cores
    chunks_tg = -(-counts.max(axis=0) // 128)              # [NTILES, NGROUPS]

    supers = []
    t0 = 0
    for ssz in SUPER_SIZES:
        supers.append(list(range(t0, t0 + ssz)))
        t0 += ssz

    slot_off = np.zeros((NTILES, NGROUPS), np.int64)       # slot start per (t,g)
    windows = []                                           # per (super, group): (slot0, nchunks)
    pos = 0
    for tiles in supers:
        for g in range(NGROUPS):
            w0 = pos
            for t in tiles:
                slot_off[t, g] = pos
                pos += int(chunks_tg[t, g]) * 128
            windows.append((w0, (pos - w0) // 128))
    S_total = pos
    C_total = S_total // 128

    idx_all = np.zeros((NCORES, S_total), np.int16)
    dst_all = np.full((NCORES, S_total), DST_SENTINEL, np.float32)
    for c in range(NCORES):
        for t in range(NTILES):
            for g in range(NGROUPS):
                n = int(counts[c, t, g])
                if n == 0:
                    continue
                s0 = int(starts[c, t, g])
                o = int(slot_off[t, g])
                idx_all[c, o:o + n] = lidx_s[s0:s0 + n]
                dst_all[c, o:o + n] = dstl_s[s0:s0 + n]
    # device layouts
    idx_dev = np.ascontiguousarray(
        np.tile(idx_all.reshape(NCORES, -1, 16).transpose(0, 2, 1), (1, 8, 1)))
    dst_dev = np.ascontiguousarray(
        dst_all.reshape(NCORES, C_total, 128).transpose(0, 2, 1)).astype(ml_dtypes.bfloat16)

    # dst-side scale and pre-divided bias, per tile
    dinv_pad = np.zeros((NCORES, PADSHARD), np.float64)
    for c in range(NCORES):
        dinv_pad[c, :SHARD] = dinv[c * SHARD:(c + 1) * SHARD]
    dinv_dev = np.ascontiguousarray(
        dinv_pad.reshape(NCORES, NTILES, 128).transpose(0, 2, 1)).astype(np.float32)
    with np.errstate(divide='ignore', invalid='ignore'):
        biasp = b_out[None, None, :] / np.where(dinv_pad > 0, dinv_pad, np.inf)[:, :, None]
    biasp_dev = np.ascontiguousarray(
        biasp.reshape(NCORES, NTILES, 128, OUT_FEAT).transpose(0, 2, 1, 3)
    ).astype(ml_dtypes.bfloat16)                           # [NC,128,NTILES,64]

    iota_np = np.tile(np.arange(128, dtype=np.float32)[None, :], (128, 1)).astype(ml_dtypes.bfloat16)
    ident_np = np.eye(128, dtype=np.float32).astype(ml_dtypes.bfloat16)

    # per-tile first/last chunk flags
    first_last = []
    for t in range(NTILES):
        gs = [g for g in range(NGROUPS) if chunks_tg[t, g] > 0]
        first_last.append((gs[0], gs[-1]) if gs else (None, None))

    meta = dict(chunks_tg=chunks_tg, supers=supers, windows=windows,
                slot_off=slot_off, S_total=S_total, C_total=C_total,
                first_last=first_last)
    per_core = dict(xT=xT, idx=idx_dev, dstv=dst_dev, dinv=dinv_dev, biasp=biasp_dev)
    consts = dict(W2=W2, iota=iota_np, ident=ident_np)
    return meta, per_core, consts


def _emit_gather_128b(nc, mybir, out_ap, in_ap, idxs_ap, num_idxs,
                      elem_size, elem_step, queue_num):
    """dma_gather with a 128-byte payload from 256B-strided rows.

    Replica of BassGpSimd.dma_gather's non-transpose path minus the
    python-level `elem_size_bytes % 256 == 0` check: the ISA only requires
    the row *stride* to be a 256B multiple (stride_bytes_256 field); the
    payload per descriptor can be any packet size.
    """
    eng = nc.gpsimd
    stride_bytes = elem_step * mybir.dt.size(in_ap.dtype)
    stride_bytes_256 = stride_bytes // 256
    assert stride_bytes % 256 == 0 and stride_bytes_256 < 256
    _in_ap = eng.lower_ap_dma(in_ap, for_custom_bir_dma=True)
    _idxs_ap = eng.lower_ap(idxs_ap)
    _out_ap = eng.lower_ap(out_ap)
    return eng.add_instruction(
        mybir.InstDMAGatherAnt(
            name=nc.get_next_instruction_name(),
            ins=[*_in_ap, _idxs_ap,
                 eng.lower_val_access(eng.to_reg(num_idxs))],
            outs=[_out_ap],
            transpose=False,
            num_idxs=num_idxs,
            elem_size=elem_size,
            stride_bytes_256=stride_bytes_256,
            gen_mode=0,
            single_packet=False,
            queue_num=queue_num,
            sbuf_tokens_per_rank=0,
            sbuf_free_dim_per_rank=0,
            sbuf_free_dim_pad_per_rank=0,
            sbuf_byte_offset=0,
        )
    )


def _build(meta):
    import concourse.bass as bass
    import concourse.tile as tile
    from concourse import bacc, mybir

    chunks_tg = meta["chunks_tg"]
    supers = meta["supers"]
    windows = meta["windows"]
    slot_off = meta["slot_off"]
    S_total = meta["S_total"]
    C_total = meta["C_total"]
    first_last = meta["first_last"]

    nc = bacc.Bacc("TRN2", target_bir_lowering=False, debug=False,
                   num_devices=NCORES, num_swdge_queues=4)
    f32, bf16, i16 = mybir.dt.float32, mybir.dt.bfloat16, mybir.dt.int16

    xT_ap = nc.dram_tensor("xt_in", [IN_FEAT, PADSHARD], f32, kind="ExternalInput").ap()
    idx_ap = nc.dram_tensor("idx_in", [128, S_total // 16], i16, kind="ExternalInput").ap()
    dst_ap = nc.dram_tensor("dst_in", [128, C_total], bf16, kind="ExternalInput").ap()
    dinv_ap = nc.dram_tensor("dinv_in", [128, NTILES], f32, kind="ExternalInput").ap()
    biasp_ap = nc.dram_tensor("biasp_in", [128, NTILES, OUT_FEAT], bf16, kind="ExternalInput").ap()
    W2_ap = nc.dram_tensor("w2_in", [IN_FEAT, OUT_FEAT], f32, kind="ExternalInput").ap()
    iota_ap = nc.dram_tensor("iota_in", [128, 128], bf16, kind="ExternalInput").ap()
    ident_ap = nc.dram_tensor("ident_in", [128, 128], bf16, kind="ExternalInput").ap()
    out_ap = nc.dram_tensor("y_out", [128, NTILES, OUT_FEAT], f32, kind="ExternalOutput").ap()

    with tile.TileContext(nc) as tc, ExitStack() as ctx:
        # g' tables in bf16, rows padded to 128 elems (256B DMA row stride);
        # upper 64 columns are never read.
        dram = ctx.enter_context(tc.tile_pool(name="dram", bufs=1, space="DRAM"))
        g_c = [dram.tile([CH_ROWS[k], 128], bf16, name=f"g_c{k}")
               for k in range(4)]
        g_full = [dram.tile([GRP_ROWS[k], 128], bf16, name=f"g_full{k}",
                            addr_space="Shared")
                  for k in range(4)]

        cpool = ctx.enter_context(tc.tile_pool(name="consts", bufs=1))
        iota_t = cpool.tile([128, 128], bf16)
        nc.sync.dma_start(iota_t[:], iota_ap[:])
        ident_t = cpool.tile([128, 128], bf16)
        nc.sync.dma_start(ident_t[:], ident_ap[:])
        dinv_t = cpool.tile([128, NTILES], f32)
        nc.sync.dma_start(dinv_t[:], dinv_ap[:])
        biasp_t = cpool.tile([128, NTILES, OUT_FEAT], bf16)
        nc.sync.dma_start(biasp_t[:], biasp_ap[:])
        w2_t = cpool.tile([128, 2, OUT_FEAT], f32)
        nc.sync.dma_start(w2_t[:], W2_ap.rearrange("(k p) f -> p k f", p=128))

        # ---- phase 1: g'_c = x'_c @ W2, 4 chunks, chunked AllGather ----
        with tc.tile_pool(name="ph1", bufs=2) as ph1, \
             tc.tile_pool(name="ph1s", bufs=4) as ph1s, \
             tc.tile_pool(name="gps", bufs=2, space="PSUM") as gps:
            for k in range(4):
                c0 = CH_START[k] * 1
                ncols = CH_ROWS[k]
                xt0 = ph1.tile([128, ncols], f32, tag="xt0")
                nc.sync.dma_start(xt0[:], xT_ap[0:128, c0:c0 + ncols])
                xt1 = ph1.tile([128, ncols], f32, tag="xt1")
                nc.sync.dma_start(xt1[:], xT_ap[128:256, c0:c0 + ncols])
                for i in range(CH_TILES[k]):
                    gp = gps.tile([128, OUT_FEAT], f32, tag="gp")
                    nc.tensor.matmul(gp[:], xt0[:, i * 128:(i + 1) * 128],
                                     w2_t[:, 0, :], start=True, stop=False)
                    nc.tensor.matmul(gp[:], xt1[:, i * 128:(i + 1) * 128],
                                     w2_t[:, 1, :], start=False, stop=True)
                    gs = ph1s.tile([128, 128], bf16, tag="gs")
                    nc.scalar.copy(gs[:, 0:OUT_FEAT], gp[:])
                    nc.vector.memset(gs[:, OUT_FEAT:128], 0.0)
                    nc.sync.dma_start(g_c[k][i * 128:(i + 1) * 128, :], gs[:])
                nc.gpsimd.collective_compute(
                    "AllGather", mybir.AluOpType.bypass,
                    ins=[g_c[k].opt()], outs=[g_full[k].opt()],
                    replica_groups=[list(range(NCORES))],
                )

        # ---- phase 2: gather + one-hot + segmented-sum matmuls ----
        p2 = ctx.enter_context(tc.tile_pool(name="p2", bufs=2))
        psum2 = ctx.enter_context(tc.tile_pool(name="ps2", bufs=2, space="PSUM"))
        outp = ctx.enter_context(tc.tile_pool(name="outp", bufs=3))

        win_i = 0
        _gq = [0]
        for si, tiles in enumerate(supers):
            gbuf = {}   # g -> (msg16, oh_win, w0)
            for g in range(NGROUPS):
                w0, wch = windows[win_i]
                win_i += 1
                if wch == 0:
                    continue
                nsl = wch * 128
                idx_t = p2.tile([128, nsl // 16], i16, tag="idx", bufs=4)
                nc.sync.dma_start(idx_t[:], idx_ap[:, w0 // 16:(w0 + nsl) // 16])
                dst_t = p2.tile([128, wch], bf16, tag="dst", bufs=4)
                nc.sync.dma_start(dst_t[:], dst_ap[:, w0 // 128:(w0 + nsl) // 128])

                # split into <=15-chunk calls: 15*128/16+1 = 121 descriptors
                # per engine fits the 128-deep SWDGE ring, so descriptor
                # generation never drain-blocks mid-call; queues rotate per
                # call so up to 4 drains overlap.
                msg16 = p2.tile([128, wch, OUT_FEAT], bf16, tag="msg16", bufs=8)
                for c0_ch in range(0, wch, 15):
                    c1_ch = min(c0_ch + 15, wch)
                    nch = c1_ch - c0_ch
                    _emit_gather_128b(
                        nc, mybir,
                        msg16[:, c0_ch:c1_ch, :], g_full[g][:, :],
                        idx_t[:, c0_ch * 8:(c0_ch + nch) * 8],
                        nch * 128, OUT_FEAT, 128, _gq[0] % 4,
                    )
                    _gq[0] += 1
                # batched 0/1 one-hot for the whole window
                oh_win = p2.tile([128, wch, 128], bf16, tag="oh", bufs=8)
                nc.vector.tensor_tensor(
                    out=oh_win[:],
                    in0=iota_t[:].unsqueeze(1).broadcast_to([128, wch, 128]),
                    in1=dst_t[:].unsqueeze(2).broadcast_to([128, wch, 128]),
                    op=mybir.AluOpType.is_equal)
                gbuf[g] = (msg16, oh_win, w0)

            stg = outp.tile([128, len(tiles), OUT_FEAT], f32, tag="stg")
            for ti, t in enumerate(tiles):
                acc = psum2.tile([128, OUT_FEAT], f32, tag=f"acc{ti % 4}",
                                 name=f"acc_{si}_{ti}")
                # bias'(t) = b_out / dinv[dst], injected via identity matmul
                nc.tensor.matmul(acc[:], ident_t[:], biasp_t[:, t, :],
                                 start=True, stop=False)
                gfirst, glast = first_last[t]
                for g in range(NGROUPS):
                    ntch = int(chunks_tg[t, g])
                    if ntch == 0:
                        continue
                    msg16, oh_win, w0 = gbuf[g]
                    c0 = (slot_off[t, g] - w0) // 128
                    for j in range(ntch):
                        cj = c0 + j
                        nc.tensor.matmul(
                            acc[:], oh_win[:, cj, :], msg16[:, cj, :],
                            start=False,
                            stop=(g == glast and j == ntch - 1),
                        )
                # flush: out = dinv[dst] * acc   (bias already inside acc)
                nc.scalar.activation(
                    stg[:, ti, :], acc[:],
                    mybir.ActivationFunctionType.Copy,
                    scale=dinv_t[:, t:t + 1])
            nc.sync.dma_start(out_ap[:, tiles[0]:tiles[0] + len(tiles), :], stg[:])

    nc.compile()
    return nc


_CACHED = {}


def make_in_maps(per_core, consts):
    in_maps = []
    for c in range(NCORES):
        in_maps.append({
            "xt_in": per_core["xT"][c],
            "idx_in": per_core["idx"][c],
            "dst_in": per_core["dstv"][c],
            "dinv_in": per_core["dinv"][c],
            "biasp_in": per_core["biasp"][c],
            "w2_in": consts["W2"],
            "iota_in": consts["iota"],
            "ident_in": consts["ident"],
        })
    return in_maps


def kernel(x, edge_index, W_gc, b_gc, W_fc, b_fc):
    from concourse import bass_utils

    meta, per_core, consts = _preprocess(x, edge_index, W_gc, b_gc, W_fc, b_fc)
    cache_key = (meta["S_total"], meta["C_total"],
                 tuple(map(tuple, meta["chunks_tg"])))
    if cache_key in _CACHED:
        nc = _CACHED[cache_key]
    else:
        nc = _build(meta)
        _CACHED.clear()
        _CACHED[cache_key] = nc

    in_maps = make_in_maps(per_core, consts)
    res = bass_utils.run_bass_kernel_spmd(nc, in_maps, core_ids=list(range(NCORES)))
    out = np.empty((N_NODES, OUT_FEAT), np.float32)
    for c in range(NCORES):
        oc = res.results[c]["y_out"]                      # [128, 98, 64]
        out[c * SHARD:(c + 1) * SHARD] = (
            oc.transpose(1, 0, 2).reshape(PADSHARD, OUT_FEAT)[:SHARD])
    return out


# revision 13
# speedup vs baseline: 1.7061x; 1.0202x over previous
"""GCN encoder kernel for 8 Trainium2 NeuronCores.

Math: out = A_hat @ (x @ (W_gc @ W_fc)) + (b_gc @ W_fc + b_fc), where
A_hat = D^-1/2 (A + I) D^-1/2 (degree over destinations incl self-loops).

Key factorization: norm(e) = dinv[src]*dinv[dst], so
  out[d] = dinv[d] * (sum_{e: dst=d} g'[src(e)]) + b_out,  g' = (dinv*x) @ W2.
The per-edge norm multiply disappears: the src factor is folded into x on the
host, the dst factor is applied once per output tile, and the bias is
injected into the PSUM accumulation as an identity matmul of b_out/dinv[d].

Distribution (1D graph parallel, dst-partitioned):
  - nodes sharded 8 ways; each core computes its shard of g' = x' @ W2 in 4
    row-chunks (x shipped pre-transposed and dinv-prescaled, so phase 1 is
    pure matmul with no PE transpose),
  - 4 chunked AllGathers (one per gather group, Shared outputs) overlap with
    phase-1 compute and phase-2 gathers of earlier groups,
  - each core processes the edges whose destination lands in its shard:
    SWDGE dma_gather of g'[src] rows (int16 indices -> 4 source groups),
    0/1 one-hot built on DVE (is_equal vs iota), msg cast f32->bf16 on the
    Scalar engine, segmented sum via PE matmuls accumulating in PSUM
    (bias'_t injected via identity matmul), flush = ACT copy scaled by
    dinv[dst] straight to the staging tile.
"""
import numpy as np
import ml_dtypes
from contextlib import ExitStack

N_NODES = 100000
IN_FEAT = 256
OUT_FEAT = 64
NCORES = 8
SHARD = N_NODES // NCORES          # 12500
NTILES = 98                        # ceil(12500/128)
PADSHARD = NTILES * 128            # 12544
CH_TILES = [25, 25, 24, 24]        # phase-1 chunks (tiles), one per group
CH_ROWS = [t * 128 for t in CH_TILES]            # [3200,3200,3072,3072]
CH_START = [0, 3200, 6400, 9472]
GRP_ROWS = [r * NCORES for r in CH_ROWS]         # [25600,25600,24576,24576]
GRP_BASE = [0, 25600, 51200, 75776]
NGROUPS = 4
SUPER_SIZES = [4] * 24 + [2]       # dst tiles per super (sum 98)
DST_SENTINEL = 255.0               # matches no iota column -> zero one-hot row


def _preprocess(x, edge_index, W_gc, b_gc, W_fc, b_fc):
    """Host-side index/weight preprocessing. Returns per-core input arrays and
    the (input-derived, core-common) chunk schedule."""
    x = np.asarray(x, np.float32)
    W2 = (np.asarray(W_gc, np.float64) @ np.asarray(W_fc, np.float64)).astype(np.float32)
    b_out = (np.asarray(b_gc, np.float64) @ np.asarray(W_fc, np.float64)
             + np.asarray(b_fc, np.float64))          # [64] f64

    src = np.asarray(edge_index[0], np.int64)
    dst = np.asarray(edge_index[1], np.int64)
    deg = np.bincount(dst, minlength=N_NODES).astype(np.float64) + 1.0
    dinv = 1.0 / np.sqrt(deg)                         # f64 [N]
    # self-loops are NOT materialized as edges: the diagonal term
    # dinv[d]*g'[d] is injected per tile from the core's own g'_c rows
    # via an identity matmul (contiguous DMA, no gather descriptors).

    # x' = dinv[v] * x[v], shipped transposed per core: [256, PADSHARD]
    xs = (x * dinv[:, None].astype(np.float32))
    xT = np.zeros((NCORES, IN_FEAT, PADSHARD), np.float32)
    for c in range(NCORES):
        xT[c, :, :SHARD] = xs[c * SHARD:(c + 1) * SHARD].T

    # source table index: group = phase-1 chunk, row = c*CH_ROWS[k] + local
    sc = src // SHARD
    sr = src % SHARD
    grp = np.searchsorted(np.array(CH_START[1:] + [PADSHARD]), sr, side='right')
    lidx = (sc * np.array(CH_ROWS)[grp] + (sr - np.array(CH_START)[grp])).astype(np.int16)

    core = dst // SHARD
    dloc = dst % SHARD
    tile = dloc // 128
    dst_local = (dloc % 128).astype(np.float32)

    # bucket edges by (core, tile, group)
    key = ((core * NTILES + tile) * NGROUPS + grp).astype(np.int64)
    order = np.argsort(key, kind="stable")
    key_s = key[order]
    lidx_s = lidx[order]
    dstl_s = dst_local[order]
    counts = np.bincount(key_s, minlength=NCORES * NTILES * NGROUPS)
    counts = counts.reshape(NCORES, NTILES, NGROUPS)
    starts = np.zeros_like(counts)
    flat = counts.reshape(NCORES, -1)
    starts.reshape(NCORES, -1)[:, 1:] = np.cumsum(flat, axis=1)[:, :-1]
    starts += np.concatenate([[0], np.cumsum(flat.sum(axis=1))[:-1]]).reshape(-1, 1, 1)

    # core-common schedule: chunks per (tile, group) = max over cores
    chunks_tg = -(-counts.max(axis=0) // 128)              # [NTILES, NGROUPS]

    supers = []
    t0 = 0
    for ssz in SUPER_SIZES:
        supers.append(list(range(t0, t0 + ssz)))
        t0 += ssz

    slot_off = np.zeros((NTILES, NGROUPS), np.int64)       # slot start per (t,g)
    windows = []                                           # per (super, group): (slot0, nchunks)
    pos = 0
    for tiles in supers:
        for g in range(NGROUPS):
            w0 = pos
            for t in tiles:
                slot_off[t, g] = pos
                pos += int(chunks_tg[t, g]) * 128
            windows.append((w0, (pos - w0) // 128))
    S_total = pos
    C_total = S_total // 128

    idx_all = np.zeros((NCORES, S_total), np.int16)
    dst_all = np.full((NCORES, S_total), DST_SENTINEL, np.float32)
    for c in range(NCORES):
        for t in range(NTILES):
            for g in range(NGROUPS):
                n = int(counts[c, t, g])
                if n == 0:
                    continue
                s0 = int(starts[c, t, g])
                o = int(slot_off[t, g])
                idx_all[c, o:o + n] = lidx_s[s0:s0 + n]
                dst_all[c, o:o + n] = dstl_s[s0:s0 + n]
    # device layouts
    idx_dev = np.ascontiguousarray(
        np.tile(idx_all.reshape(NCORES, -1, 16).transpose(0, 2, 1), (1, 8, 1)))
    dst_dev = np.ascontiguousarray(
        dst_all.reshape(NCORES, C_total, 128).transpose(0, 2, 1)).astype(ml_dtypes.bfloat16)

    # dst-side scale and pre-divided bias, per tile
    dinv_pad = np.zeros((NCORES, PADSHARD), np.float64)
    for c in range(NCORES):
        dinv_pad[c, :SHARD] = dinv[c * SHARD:(c + 1) * SHARD]
    dinv_dev = np.ascontiguousarray(
        dinv_pad.reshape(NCORES, NTILES, 128).transpose(0, 2, 1)).astype(np.float32)
    with np.errstate(divide='ignore', invalid='ignore'):
        biasp = b_out[None, None, :] / np.where(dinv_pad > 0, dinv_pad, np.inf)[:, :, None]
    biasp_dev = np.ascontiguousarray(
        biasp.reshape(NCORES, NTILES, 128, OUT_FEAT).transpose(0, 2, 1, 3)
    ).astype(ml_dtypes.bfloat16)                           # [NC,128,NTILES,64]

    iota_np = np.tile(np.arange(128, dtype=np.float32)[None, :], (128, 1)).astype(ml_dtypes.bfloat16)
    ident_np = np.eye(128, dtype=np.float32).astype(ml_dtypes.bfloat16)

    # per-tile first/last chunk flags
    first_last = []
    for t in range(NTILES):
        gs = [g for g in range(NGROUPS) if chunks_tg[t, g] > 0]
        first_last.append((gs[0], gs[-1]) if gs else (None, None))

    meta = dict(chunks_tg=chunks_tg, supers=supers, windows=windows,
                slot_off=slot_off, S_total=S_total, C_total=C_total,
                first_last=first_last)
    per_core = dict(xT=xT, idx=idx_dev, dstv=dst_dev, dinv=dinv_dev, biasp=biasp_dev)
    consts = dict(W2=W2, iota=iota_np, ident=ident_np)
    return meta, per_core, consts


def _emit_gather_128b(nc, mybir, out_ap, in_ap, idxs_ap, num_idxs,
                      elem_size, elem_step, queue_num):
    """dma_gather with a 128-byte payload from 256B-strided rows.

    Replica of BassGpSimd.dma_gather's non-transpose path minus the
    python-level `elem_size_bytes % 256 == 0` check: the ISA only requires
    the row *stride* to be a 256B multiple (stride_bytes_256 field); the
    payload per descriptor can be any packet size.
    """
    eng = nc.gpsimd
    stride_bytes = elem_step * mybir.dt.size(in_ap.dtype)
    stride_bytes_256 = stride_bytes // 256
    assert stride_bytes % 256 == 0 and stride_bytes_256 < 256
    _in_ap = eng.lower_ap_dma(in_ap, for_custom_bir_dma=True)
    _idxs_ap = eng.lower_ap(idxs_ap)
    _out_ap = eng.lower_ap(out_ap)
    return eng.add_instruction(
        mybir.InstDMAGatherAnt(
            name=nc.get_next_instruction_name(),
            ins=[*_in_ap, _idxs_ap,
                 eng.lower_val_access(eng.to_reg(num_idxs))],
            outs=[_out_ap],
            transpose=False,
            num_idxs=num_idxs,
            elem_size=elem_size,
            stride_bytes_256=stride_bytes_256,
            gen_mode=0,
            single_packet=False,
            queue_num=queue_num,
            sbuf_tokens_per_rank=0,
            sbuf_free_dim_per_rank=0,
            sbuf_free_dim_pad_per_rank=0,
            sbuf_byte_offset=0,
        )
    )


def _build(meta):
    import concourse.bass as bass
    import concourse.tile as tile
    from concourse import bacc, mybir

    chunks_tg = meta["chunks_tg"]
    supers = meta["supers"]
    windows = meta["windows"]
    slot_off = meta["slot_off"]
    S_total = meta["S_total"]
    C_total = meta["C_total"]
    first_last = meta["first_last"]

    nc = bacc.Bacc("TRN2", target_bir_lowering=False, debug=False,
                   num_devices=NCORES, num_swdge_queues=4)
    f32, bf16, i16 = mybir.dt.float32, mybir.dt.bfloat16, mybir.dt.int16

    xT_ap = nc.dram_tensor("xt_in", [IN_FEAT, PADSHARD], f32, kind="ExternalInput").ap()
    idx_ap = nc.dram_tensor("idx_in", [128, S_total // 16], i16, kind="ExternalInput").ap()
    dst_ap = nc.dram_tensor("dst_in", [128, C_total], bf16, kind="ExternalInput").ap()
    dinv_ap = nc.dram_tensor("dinv_in", [128, NTILES], f32, kind="ExternalInput").ap()
    biasp_ap = nc.dram_tensor("biasp_in", [128, NTILES, OUT_FEAT], bf16, kind="ExternalInput").ap()
    W2_ap = nc.dram_tensor("w2_in", [IN_FEAT, OUT_FEAT], f32, kind="ExternalInput").ap()
    iota_ap = nc.dram_tensor("iota_in", [128, 128], bf16, kind="ExternalInput").ap()
    ident_ap = nc.dram_tensor("ident_in", [128, 128], bf16, kind="ExternalInput").ap()
    out_ap = nc.dram_tensor("y_out", [128, NTILES, OUT_FEAT], f32, kind="ExternalOutput").ap()

    with tile.TileContext(nc) as tc, ExitStack() as ctx:
        # g' tables in bf16, rows padded to 128 elems (256B DMA row stride);
        # upper 64 columns are never read.
        dram = ctx.enter_context(tc.tile_pool(name="dram", bufs=1, space="DRAM"))
        g_c = [dram.tile([CH_ROWS[k], 128], bf16, name=f"g_c{k}")
               for k in range(4)]
        g_full = [dram.tile([GRP_ROWS[k], 128], bf16, name=f"g_full{k}",
                            addr_space="Shared")
                  for k in range(4)]

        cpool = ctx.enter_context(tc.tile_pool(name="consts", bufs=1))
        iota_t = cpool.tile([128, 128], bf16)
        nc.sync.dma_start(iota_t[:], iota_ap[:])
        ident_t = cpool.tile([128, 128], bf16)
        nc.sync.dma_start(ident_t[:], ident_ap[:])
        dinv_t = cpool.tile([128, NTILES], f32)
        nc.sync.dma_start(dinv_t[:], dinv_ap[:])
        biasp_t = cpool.tile([128, NTILES, OUT_FEAT], bf16)
        nc.sync.dma_start(biasp_t[:], biasp_ap[:])
        w2_t = cpool.tile([128, 2, OUT_FEAT], f32)
        nc.sync.dma_start(w2_t[:], W2_ap.rearrange("(k p) f -> p k f", p=128))

        # ---- phase 1: g'_c = x'_c @ W2, 4 chunks, chunked AllGather ----
        with tc.tile_pool(name="ph1", bufs=2) as ph1, \
             tc.tile_pool(name="ph1s", bufs=4) as ph1s, \
             tc.tile_pool(name="gps", bufs=2, space="PSUM") as gps:
            for k in range(4):
                c0 = CH_START[k] * 1
                ncols = CH_ROWS[k]
                xt0 = ph1.tile([128, ncols], f32, tag="xt0")
                nc.sync.dma_start(xt0[:], xT_ap[0:128, c0:c0 + ncols])
                xt1 = ph1.tile([128, ncols], f32, tag="xt1")
                nc.sync.dma_start(xt1[:], xT_ap[128:256, c0:c0 + ncols])
                for i in range(CH_TILES[k]):
                    gp = gps.tile([128, OUT_FEAT], f32, tag="gp")
                    nc.tensor.matmul(gp[:], xt0[:, i * 128:(i + 1) * 128],
                                     w2_t[:, 0, :], start=True, stop=False)
                    nc.tensor.matmul(gp[:], xt1[:, i * 128:(i + 1) * 128],
                                     w2_t[:, 1, :], start=False, stop=True)
                    gs = ph1s.tile([128, 128], bf16, tag="gs")
                    nc.scalar.copy(gs[:, 0:OUT_FEAT], gp[:])
                    nc.vector.memset(gs[:, OUT_FEAT:128], 0.0)
                    nc.sync.dma_start(g_c[k][i * 128:(i + 1) * 128, :], gs[:])
                nc.gpsimd.collective_compute(
                    "AllGather", mybir.AluOpType.bypass,
                    ins=[g_c[k].opt()], outs=[g_full[k].opt()],
                    replica_groups=[list(range(NCORES))],
                )

        # ---- phase 2: gather + one-hot + segmented-sum matmuls ----
        p2 = ctx.enter_context(tc.tile_pool(name="p2", bufs=2))
        psum2 = ctx.enter_context(tc.tile_pool(name="ps2", bufs=2, space="PSUM"))
        outp = ctx.enter_context(tc.tile_pool(name="outp", bufs=3))

        win_i = 0
        _gq = [0]
        for si, tiles in enumerate(supers):
            gbuf = {}   # g -> (msg16, oh_win, w0)
            for g in range(NGROUPS):
                w0, wch = windows[win_i]
                win_i += 1
                if wch == 0:
                    continue
                nsl = wch * 128
                idx_t = p2.tile([128, nsl // 16], i16, tag="idx", bufs=4)
                nc.sync.dma_start(idx_t[:], idx_ap[:, w0 // 16:(w0 + nsl) // 16])
                dst_t = p2.tile([128, wch], bf16, tag="dst", bufs=4)
                nc.sync.dma_start(dst_t[:], dst_ap[:, w0 // 128:(w0 + nsl) // 128])

                # split into <=15-chunk calls: 15*128/16+1 = 121 descriptors
                # per engine fits the 128-deep SWDGE ring, so descriptor
                # generation never drain-blocks mid-call; queues rotate per
                # call so up to 4 drains overlap.
                msg16 = p2.tile([128, wch, OUT_FEAT], bf16, tag="msg16", bufs=8)
                for c0_ch in range(0, wch, 15):
                    c1_ch = min(c0_ch + 15, wch)
                    nch = c1_ch - c0_ch
                    _emit_gather_128b(
                        nc, mybir,
                        msg16[:, c0_ch:c1_ch, :], g_full[g][:, :],
                        idx_t[:, c0_ch * 8:(c0_ch + nch) * 8],
                        nch * 128, OUT_FEAT, 128, _gq[0] % 4,
                    )
                    _gq[0] += 1
                # batched 0/1 one-hot for the whole window
                oh_win = p2.tile([128, wch, 128], bf16, tag="oh", bufs=8)
                nc.vector.tensor_tensor(
                    out=oh_win[:],
                    in0=iota_t[:].unsqueeze(1).broadcast_to([128, wch, 128]),
                    in1=dst_t[:].unsqueeze(2).broadcast_to([128, wch, 128]),
                    op=mybir.AluOpType.is_equal)
                gbuf[g] = (msg16, oh_win, w0)

            stg = outp.tile([128, len(tiles), OUT_FEAT], f32, tag="stg")
            for ti, t in enumerate(tiles):
                # own-shard g' rows for the self-loop (diagonal) term
                kc = 0
                while t >= sum(CH_TILES[:kc + 1]):
                    kc += 1
                trel = t - sum(CH_TILES[:kc])
                gself = p2.tile([128, 128], bf16, tag="gself", bufs=4)
                nc.sync.dma_start(
                    gself[:], g_c[kc][trel * 128:(trel + 1) * 128, :])
                acc = psum2.tile([128, OUT_FEAT], f32, tag=f"acc{ti % 4}",
                                 name=f"acc_{si}_{ti}")
                # bias'(t) = b_out / dinv[dst], injected via identity matmul
                nc.tensor.matmul(acc[:], ident_t[:], biasp_t[:, t, :],
                                 start=True, stop=False)
                # self-loop: acc[d] += g'[d]
                nc.tensor.matmul(acc[:], ident_t[:], gself[:, 0:OUT_FEAT],
                                 start=False, stop=False)
                gfirst, glast = first_last[t]
                for g in range(NGROUPS):
                    ntch = int(chunks_tg[t, g])
                    if ntch == 0:
                        continue
                    msg16, oh_win, w0 = gbuf[g]
                    c0 = (slot_off[t, g] - w0) // 128
                    for j in range(ntch):
                        cj = c0 + j
                        nc.tensor.matmul(
                            acc[:], oh_win[:, cj, :], msg16[:, cj, :],
                            start=False,
                            stop=(g == glast and j == ntch - 1),
                        )
                # flush: out = dinv[dst] * acc   (bias already inside acc)
                nc.scalar.activation(
                    stg[:, ti, :], acc[:],
                    mybir.ActivationFunctionType.Copy,
                    scale=dinv_t[:, t:t + 1])
            nc.sync.dma_start(out_ap[:, tiles[0]:tiles[0] + len(tiles), :], stg[:])

    nc.compile()
    return nc


_CACHED = {}


def make_in_maps(per_core, consts):
    in_maps = []
    for c in range(NCORES):
        in_maps.append({
            "xt_in": per_core["xT"][c],
            "idx_in": per_core["idx"][c],
            "dst_in": per_core["dstv"][c],
            "dinv_in": per_core["dinv"][c],
            "biasp_in": per_core["biasp"][c],
            "w2_in": consts["W2"],
            "iota_in": consts["iota"],
            "ident_in": consts["ident"],
        })
    return in_maps


def kernel(x, edge_index, W_gc, b_gc, W_fc, b_fc):
    from concourse import bass_utils

    meta, per_core, consts = _preprocess(x, edge_index, W_gc, b_gc, W_fc, b_fc)
    cache_key = (meta["S_total"], meta["C_total"],
                 tuple(map(tuple, meta["chunks_tg"])))
    if cache_key in _CACHED:
        nc = _CACHED[cache_key]
    else:
        nc = _build(meta)
        _CACHED.clear()
        _CACHED[cache_key] = nc

    in_maps = make_in_maps(per_core, consts)
    res = bass_utils.run_bass_kernel_spmd(nc, in_maps, core_ids=list(range(NCORES)))
    out = np.empty((N_NODES, OUT_FEAT), np.float32)
    for c in range(NCORES):
        oc = res.results[c]["y_out"]                      # [128, 98, 64]
        out[c * SHARD:(c + 1) * SHARD] = (
            oc.transpose(1, 0, 2).reshape(PADSHARD, OUT_FEAT)[:SHARD])
    return out
